# revision 26
# baseline (speedup 1.0000x reference)
"""Causal multi-head attention with RoPE on 8 Trainium2 NeuronCores.

Problem: B=2, S=2048, D=1024, H=16 heads, head_dim=64, fp32 in/out.

Sharding (hardcoded): 8 cores = 2 (batch) x 4 (head groups of 4 heads).
Core c handles batch b = c // 4 and heads [hg*4, hg*4+4), hg = c % 4.
Each core computes its 4 heads' attention plus the partial output
projection o_part = attn_part @ wo[:, cols].T; the host sums the 4
partials per batch (the row-parallel reduction) to form the output.

Device dataflow per core (matmuls in bf16, fp32 accumulation):
  qT/kT projections in transposed layout (channels on partitions),
  RoPE applied in that layout: channels of wq/wk are pre-permuted on
  host so each head's dims are [evens, odds]. Rotation is
    rot = raw*cos + P@(raw*sin_sw)
  with P a 128x128 half-swap permutation matmul and sin_sw the
  row-swapped sign-baked sin table, so both elementwise multiplies are
  all-bf16 (2x DVE rate) and the perm matmul's moving operand is the
  already-scaled u = raw*sin_sw.
  Scores are computed transposed, sT = k_rot @ q_rot.T (Sk on
  partitions), exp applied on ScalarE (scale=1/8 folded in), causal
  masking via a fused triangular-mask multiply on the diagonal
  128-block (both heads in one op, on the Pool/GPSIMD engine, which is
  otherwise idle). A@V uses exp(sT) blocks as the moving operand with
  stationary [v_h | ones] (M=65), so partition 64 of the accumulator
  carries the softmax denominators. Normalization broadcasts 1/r
  across partitions with a single fp32r ones-matmul (fp32r moving is
  full-rate for free dim >= 256), then multiplies straight out of the
  two PSUM tiles. The wo projection contracts the 256 channels into a
  [128,1024] PSUM tile per seq block and stages bf16 results to DRAM;
  the host sums partials in fp32.
"""

import numpy as np
import ml_dtypes

import concourse.bass as bass
import concourse.mybir as mybir
import concourse.tile as tile_mod
from concourse.bass_utils import run_bass_kernel_spmd

BF16 = ml_dtypes.bfloat16
dt = mybir.dt

B = 2
S = 2048
D = 1024
H = 16
HD = 64          # head dim
HPC = 4          # heads per core
NCH = HPC * HD   # 256 channels per core
KT = D // 128    # 8 contraction tiles over D
NM = S // 128    # 16 seq tiles of 128
NJ = S // 512    # 4 seq chunks of 512
THETA = 10000.0

_CACHE = {}
_MM_LOG = []  # one label per emitted PE matmul/ldweights site, for trace attribution

# Bumped on every kernel change: the Neuron compile cache hashes the HLO
# module WITHOUT the embedded BIR payload, so two different kernels with
# identical I/O signatures collide. A version-sized dummy input forces a
# distinct hash per kernel revision.
KVER = 15


def _split_multi_waits(nc):
    # This container's walrus build rejects >1 sync wait per instruction.
    # Hoist extra waits onto InstEventSemaphore carriers placed before the
    # instruction in the same engine's stream.
    for bb in nc.main_func.blocks:
        new_list = []
        for ins in bb.instructions:
            si = getattr(ins, "sync_info", None)
            if si is not None and si.on_wait and len(si.on_wait) > 1:
                waits = list(si.on_wait)
                si.on_wait = [waits[-1]]
                for w in waits[:-1]:
                    ev = mybir.InstEventSemaphore(
                        name=nc.get_next_instruction_name(),
                        engine=ins.engine,
                        ins=[],
                        outs=[],
                        sync_info=mybir.SyncInfo(on_wait=[w], on_update=[]),
                    )
                    nc.register_instruction(ev, overwrite=True)
                    new_list.append(ev)
            new_list.append(ins)
        bb.instructions[:] = new_list


def _build_nc():
    nc = bass.Bass("TRN2", target_bir_lowering=False)
    _MM_LOG.clear()

    _raw_mm = nc.tensor.matmul

    def _mm(label, *args, **kw):
        _MM_LOG.append(label)
        return _raw_mm(*args, **kw)

    # Inputs are shipped in SBUF layout (128 partitions first).
    xT = nc.dram_tensor("xT", [KT * 128, S], dt.bfloat16, kind="ExternalInput")
    # packed bf16: wv|wq|wk (3*2048) | perm (128) | tri (128) | wo (2048)
    wpack = nc.dram_tensor("wpack", [128, 8448], dt.bfloat16, kind="ExternalInput")
    # packed bf16: cos (2048) | sin_sw (2048) | ver pad (KVER)
    fpack = nc.dram_tensor("fpack", [128, 4096 + KVER], dt.bfloat16,
                           kind="ExternalInput")
    onesr = nc.dram_tensor("onesr", [1, 64], dt.float32r, kind="ExternalInput")
    out = nc.dram_tensor("o", [S, D], dt.bfloat16, kind="ExternalOutput")

    EXP = mybir.ActivationFunctionType.Exp

    with tile_mod.TileContext(nc) as tc:
        with (
            tc.tile_pool(name="io", bufs=1) as io,
            tc.tile_pool(name="wk1", bufs=5) as wkp,
            tc.tile_pool(name="ep_d", bufs=4) as ep_d,
            tc.tile_pool(name="ep_s", bufs=4) as ep_s,
            tc.tile_pool(name="sm", bufs=4) as sm,
            tc.tile_pool(name="ob", bufs=4) as ob,
            tc.tile_pool(name="ps", bufs=2, space="PSUM") as ps,
            tc.tile_pool(name="pscr", bufs=2, space="PSUM") as scr_p,
            tc.tile_pool(name="po", bufs=2, space="PSUM") as po_p,
        ):
            # Input DMAs split across the three HWDGE queues (SP / Act /
            # DVE) so the first projection matmuls start ~7us in instead of
            # waiting behind one serial queue.
            def _ld(eng, name, dram_slice, cols):
                t = io.tile([128, cols], dt.bfloat16, tag=name, name=name)
                eng.dma_start(t[:], dram_slice)
                return t

            # xT (and the RoPE tables) stream in 512-column j-chunks: the
            # j=0 work needs only 1 MB before compute starts. Three queues:
            # SP + Act HWDGE plus the gpsimd SWDGE for the small tables.
            xTs = [
                io.tile([128, S], dt.bfloat16, tag=f"xT{k}", name=f"xT{k}")
                for k in range(KT)
            ]
            cos_sb = io.tile([128, S], dt.bfloat16, tag="cos", name="cos")
            sin_sb = io.tile([128, S], dt.bfloat16, tag="sin", name="sin")

            # the sim serializes all transfers on one DMA resource and all
            # descriptor-gen on one HWDGE resource (~630 ns per dma_start),
            # so use few >=256 KB DMAs in dependency-critical order
            def _ld_half(eng, t, dram, r0, src_c, dst_c):
                eng.dma_start(
                    t[:, dst_c : dst_c + 1024],
                    dram[r0 : r0 + 128, src_c : src_c + 1024],
                )

            wq_sb = _ld(nc.sync, "wq", wpack[:, 2048:4096], 2048)
            for k in range(KT):
                _ld_half(nc.sync, xTs[k], xT, k * 128, 0, 0)
            wk_sb = _ld(nc.sync, "wk", wpack[:, 4096:6144], 2048)
            _ld_half(nc.sync, sin_sb, fpack, 0, 2048, 0)
            _ld_half(nc.sync, cos_sb, fpack, 0, 0, 0)
            misc_sb = _ld(nc.sync, "misc", wpack[:, 6144:6400], 256)
            perm_sb = misc_sb[:, 0:128]
            tri_sb = misc_sb[:, 128:256]
            wv_sb = _ld(nc.sync, "wv", wpack[:, 0:2048], 2048)
            for k in range(KT):
                _ld_half(nc.sync, xTs[k], xT, k * 128, 1024, 1024)
            _ld_half(nc.sync, sin_sb, fpack, 0, 3072, 1024)
            _ld_half(nc.sync, cos_sb, fpack, 0, 1024, 1024)
            wo_sb = _ld(nc.sync, "wo", wpack[:, 6400:8448], 2048)

            ones_sb = io.tile([1, 64], dt.float32r, tag="ones", name="ones_sb")
            nc.sync.dma_start(ones_sb[:], onesr[:])

            # fine-grained persistent tiles: precise cross-phase dependencies
            q_t = [
                [io.tile([128, 512], dt.bfloat16, tag=f"q{g}{j}", name=f"q{g}{j}")
                 for j in range(NJ)] for g in range(2)
            ]
            k_t = [
                [io.tile([128, 512], dt.bfloat16, tag=f"k{g}{j}", name=f"k{g}{j}")
                 for j in range(NJ)] for g in range(2)
            ]
            v_t = [
                io.tile([128, HPC * 65], dt.bfloat16, tag=f"v{m}", name=f"v{m}")
                for m in range(NM)
            ]
            attn_t = [
                [io.tile([128, 512], dt.bfloat16, tag=f"at{g}{j}", name=f"at{g}{j}")
                 for j in range(NJ)] for g in range(2)
            ]
            # denominator ones columns never change: set them once up front
            for m in range(NM):
                blk = v_t[m][:].rearrange("p (h c) -> p h c", c=65)
                nc.vector.memset(blk[:, :, 64:65], 1.0)

            # ---- work quanta: each closure emits ~1-2 PE matmuls (plus
            # their DVE/Act side ops). The attention block loop pops one
            # quantum per block so PE always has independent work queued
            # during the exp / normalize / copy latencies. ----

            def v_quanta(m):
                st = {}

                def q1():
                    st["pv"] = ps.tile([128, NCH], dt.float32, tag="ps", name="pv")
                    for k in range(4):
                        _mm(
                            f"v{m}k{k}",
                            st["pv"][:],
                            xTs[k][:, m * 128 : (m + 1) * 128],
                            wv_sb[:, k * NCH : (k + 1) * NCH],
                            start=(k == 0),
                            stop=False,
                        )

                def q2():
                    for k in range(4, KT):
                        _mm(
                            f"v{m}k{k}",
                            st["pv"][:],
                            xTs[k][:, m * 128 : (m + 1) * 128],
                            wv_sb[:, k * NCH : (k + 1) * NCH],
                            start=False,
                            stop=(k == KT - 1),
                        )
                    blk = v_t[m][:].rearrange("p (h c) -> p h c", c=65)
                    nc.scalar.copy(
                        blk[:, :, 0:64], st["pv"][:].rearrange("p (h c) -> p h c", c=64)
                    )

                return [q1, q2]

            def qk_quanta(dst_t, w_sb, g, j):
                st = {}

                def mmk(k0, k1):
                    def f():
                        if "pp" not in st:
                            st["pp"] = ps.tile(
                                [128, 512], dt.float32, tag="ps", name="pp"
                            )
                        for k in range(k0, k1):
                            _mm(
                                f"qk{g}j{j}k{k}",
                                st["pp"][:],
                                w_sb[:, k * NCH + g * 128 : k * NCH + (g + 1) * 128],
                                xTs[k][:, j * 512 : (j + 1) * 512],
                                start=(k == 0),
                                stop=(k == KT - 1),
                            )

                    return f

                def rope():
                    pp = st["pp"]
                    raw = wkp.tile([128, 512], dt.bfloat16, tag="raw", name="raw")
                    nc.vector.tensor_copy(raw[:], pp[:])
                    u = wkp.tile([128, 512], dt.bfloat16, tag="u", name="u")
                    nc.vector.tensor_mul(
                        u[:], raw[:], sin_sb[:, j * 512 : (j + 1) * 512]
                    )
                    t1 = wkp.tile([128, 512], dt.bfloat16, tag="t1", name="t1")
                    nc.vector.tensor_mul(
                        t1[:], raw[:], cos_sb[:, j * 512 : (j + 1) * 512]
                    )
                    st["u"], st["t1"] = u, t1

                def perm():
                    pq = st["pp"]
                    _mm(f"perm{g}j{j}", pq[:], perm_sb, st["u"][:], start=True, stop=True)
                    nc.vector.tensor_add(dst_t[g][j][:], st["t1"][:], pq[:])

                return [mmk(0, 2), mmk(2, 4), mmk(4, 6), mmk(6, 8), rope, perm]

            pos_store = {}

            def norm_quanta(hp, j):
                g = hp

                def q1():
                    pos = pos_store[(hp, j)]
                    recs, us = [], []
                    for t in range(2):
                        rec = sm.tile([1, 512], dt.float32r, tag="rec", name="rec")
                        with nc.allow_low_precision(reason="f32r == fp32 bits"):
                            nc.vector.reciprocal(rec[:], pos[t][64:65, :])
                        recs.append(rec)
                        # numerators to SBUF: the normalize mul may read only
                        # one operand from PSUM (hardware limit), and pb must
                        # stay there (matmul output)
                        u = sm.tile([64, 512], dt.bfloat16, tag="u65", name="u65")
                        nc.vector.tensor_copy(u[:], pos[t][0:64, :])
                        us.append(u)
                    pos_store[("rec", hp, j)] = recs
                    pos_store[("us", hp, j)] = us
                    pb = ps.tile([64, 512], dt.float32, tag="ps", name="pb")
                    _mm(f"bc{hp}j{j}t0", pb[:], ones_sb[:], recs[0][:], start=True, stop=True)
                    pos_store[("pb", hp, j)] = pb

                def q2():
                    recs = pos_store[("rec", hp, j)]
                    us = pos_store[("us", hp, j)]
                    pb0 = pos_store[("pb", hp, j)]
                    pb1 = ps.tile([64, 512], dt.float32, tag="ps", name="pb")
                    _mm(f"bc{hp}j{j}t1", pb1[:], ones_sb[:], recs[1][:], start=True, stop=True)
                    nc.vector.tensor_mul(attn_t[g][j][0:64, :], us[0][:], pb0[:])
                    nc.vector.tensor_mul(attn_t[g][j][64:128, :], us[1][:], pb1[:])

                return [q1, q2]

            def wo_quanta(m):
                st = {}

                def qn(n):
                    def f():
                        if "osb" not in st:
                            st["osb"] = ob.tile([128, 1024], dt.bfloat16, tag="osb", name="osb")
                        pf = ps.tile([128, 512], dt.float32, tag="ps", name=f"pf{n}")
                        for g in range(2):
                            _mm(
                                f"wo{m}n{n}g{g}",
                                pf[:],
                                attn_t[g][m // 4][:, (m % 4) * 128 : (m % 4 + 1) * 128],
                                wo_sb[:, g * D + n * 512 : g * D + (n + 1) * 512],
                                start=(g == 0),
                                stop=(g == 1),
                            )
                        nc.vector.tensor_copy(
                            st["osb"][:, n * 512 : (n + 1) * 512], pf[:]
                        )
                        if n == 1:
                            nc.scalar.dma_start(
                                out[m * 128 : (m + 1) * 128, :], st["osb"][:]
                            )

                    return f

                return [qn(0), qn(1)]

            # ---- attention: sT = k_rot @ q_rot.T, exp, A@V with denominators.
            # Diagonal blocks (masked on Pool) run first with their A@Vs
            # deferred to the end, so the Pool mask latency hides behind the
            # sub-diagonal stream. One filler quantum runs per block. ----
            def attention(hp, j, fillers):
                g = hp
                nblk = 4 * j + 4
                pos = [
                    po_p.tile([65, 512], dt.float32, tag="po", name=f"po{t}")
                    for t in range(2)
                ]
                pos_store[(hp, j)] = pos

                def fill(n=1):
                    for _ in range(n):
                        if fillers:
                            fillers.pop(0)()

                def scores(i, lo, diag):
                    psw = scr_p.tile([128, 1024], dt.float32, tag="pscr", name="psw")
                    for t in range(2):
                        off = 64 * t
                        _mm(
                            f"sc{hp}j{j}i{i}t{t}",
                            psw[:, t * 512 + lo : (t + 1) * 512],
                            k_t[g][i // 4][off : off + 64, (i % 4) * 128 : (i % 4 + 1) * 128],
                            q_t[g][j][off : off + 64, lo:512],
                            start=True,
                            stop=True,
                        )
                    e = (ep_d if diag else ep_s).tile([128, 1024], dt.bfloat16, tag="e", name="e")
                    if lo == 0:
                        nc.scalar.activation(e[:], psw[:], EXP, scale=0.125)
                    else:
                        src_ap = psw[:].rearrange("p (t c) -> p t c", t=2)[:, :, lo:512]
                        dst_ap = e[:].rearrange("p (t c) -> p t c", t=2)[:, :, lo:512]
                        nc.scalar.activation(dst_ap, src_ap, EXP, scale=0.125)
                    if diag:
                        eview = e[:].rearrange("p (t c) -> p t c", t=2)
                        tri_b = bass.AP(
                            tri_sb.tensor,
                            tri_sb.offset,
                            [tri_sb.ap[0], [0, 2], tri_sb.ap[1]],
                        )
                        nc.gpsimd.tensor_mul(
                            eview[:, :, lo : lo + 128],
                            eview[:, :, lo : lo + 128],
                            tri_b,
                        )
                    return e

                av_n = [0]

                def av(i, e, lo):
                    for t in range(2):
                        h = 2 * hp + t
                        _mm(
                            f"av{hp}j{j}i{i}t{t}",
                            pos[t][0:65, lo:512],
                            v_t[i][:, h * 65 : (h + 1) * 65],
                            e[:, t * 512 + lo : (t + 1) * 512],
                            start=(av_n[0] == 0),
                            stop=(av_n[0] == nblk - 1),
                        )
                    av_n[0] += 1

                # phase A: diagonal blocks, A@V deferred
                diag_es = []
                for r in range(4):
                    i = 4 * j + r
                    lo = 128 * r
                    diag_es.append((i, scores(i, lo, True), lo))
                    fill()
                fill(2)
                # phase B: sub-diagonal stream
                for i in range(4 * j):
                    e = scores(i, 0, False)
                    fill()
                    av(i, e, 0)
                # phase C: deferred diagonal A@Vs
                for i, e, lo in diag_es:
                    fill()
                    av(i, e, lo)
                # drain whatever didn't fit between blocks
                while fillers:
                    fillers.pop(0)()

            # ---- schedule: prologue direct, then attention calls with the
            # next group's projection / normalize / output work as fillers ----
            def run_all(quanta):
                for q in quanta:
                    q()

            run_all(qk_quanta(q_t, wq_sb, 0, 0))
            run_all(qk_quanta(k_t, wk_sb, 0, 0))
            for m in range(4):
                run_all(v_quanta(m))
            run_all(qk_quanta(q_t, wq_sb, 1, 0))
            attention(0, 0, qk_quanta(k_t, wk_sb, 1, 0))
            f = norm_quanta(0, 0) + qk_quanta(q_t, wq_sb, 0, 1) \
                + qk_quanta(k_t, wk_sb, 0, 1)
            for m in range(4, 8):
                f += v_quanta(m)
            attention(1, 0, f)
            for j in range(1, NJ):
                f = norm_quanta(1, j - 1) + qk_quanta(q_t, wq_sb, 1, j) \
                    + qk_quanta(k_t, wk_sb, 1, j)
                for m in range(4 * (j - 1), 4 * j):
                    f += wo_quanta(m)
                attention(0, j, f)
                f = norm_quanta(0, j)
                if j + 1 < NJ:
                    f += qk_quanta(q_t, wq_sb, 0, j + 1)
                    f += qk_quanta(k_t, wk_sb, 0, j + 1)
                    for m in range(4 * (j + 1), 4 * (j + 2)):
                        f += v_quanta(m)
                attention(1, j, f)
            run_all(norm_quanta(1, NJ - 1))
            for m in range(12, 16):
                run_all(wo_quanta(m))

    _split_multi_waits(nc)
    return nc


def _sbuf_layout(a128xN):
    # (T*128, N) -> (128, T*N) with tile t at columns [t*N, (t+1)*N)
    t = a128xN.shape[0] // 128
    n = a128xN.shape[1]
    return np.ascontiguousarray(
        a128xN.reshape(t, 128, n).transpose(1, 0, 2).reshape(128, t * n)
    )


def _host_prep(x, wq, wk, wv, wo, token_positions):
    x = np.asarray(x, dtype=np.float32)
    wq = np.asarray(wq, dtype=np.float32)
    wk = np.asarray(wk, dtype=np.float32)
    wv = np.asarray(wv, dtype=np.float32)
    wo = np.asarray(wo, dtype=np.float32)
    pos = np.asarray(token_positions).astype(np.float32)

    # deinterleave channel order within each head for q/k: [evens, odds]
    de = np.concatenate([np.arange(0, HD, 2), np.arange(1, HD, 2)])

    # RoPE tables, extended to the 128-partition tile layout
    inv_freq = (1.0 / (THETA ** (np.arange(0, HD, 2, dtype=np.float32) / HD))).astype(
        np.float32
    )
    freqs = pos[:, None] * inv_freq[None, :]  # (S, 32)
    cosT = np.cos(freqs).astype(np.float32).T  # (32, S)
    sinT = np.sin(freqs).astype(np.float32).T
    cos_l = np.ascontiguousarray(np.tile(cosT, (4, 1)))  # (128, S)
    # row-swapped sin with signs baked: u = raw*sin_sw is built BEFORE the
    # half-swap matmul, so the table rows live at the pre-swap positions
    sin_sw = np.ascontiguousarray(
        np.concatenate([sinT, -sinT, sinT, -sinT], axis=0)
    )

    # 128x128 half-swap permutation (block diag of two 64-blocks)
    p64 = np.zeros((64, 64), np.float32)
    for i in range(64):
        p64[i, (i + 32) % 64] = 1.0
    perm_l = np.zeros((128, 128), np.float32)
    perm_l[:64, :64] = p64
    perm_l[64:, 64:] = p64

    tri_l = (np.arange(128)[None, :] >= np.arange(128)[:, None]).astype(np.float32)

    in_maps = []
    for c in range(8):
        b, hg = divmod(c, 4)
        rows = hg * NCH + np.arange(NCH)
        # per-head deinterleave for q/k channel rows
        rows_de = (rows.reshape(HPC, HD)[:, de]).reshape(-1)

        xT = np.ascontiguousarray(x[b].T)  # (D, S)
        wq_t = np.ascontiguousarray(wq[rows_de, :].T)  # (D, 256)
        wk_t = np.ascontiguousarray(wk[rows_de, :].T)
        wv_t = np.ascontiguousarray(wv[rows, :].T)
        wo_t = np.ascontiguousarray(wo[:, rows].T)  # (256, D)

        wpk = np.concatenate(
            [
                _sbuf_layout(wv_t),
                _sbuf_layout(wq_t),
                _sbuf_layout(wk_t),
                perm_l,
                tri_l,
                _sbuf_layout(wo_t),
            ],
            axis=1,
        ).astype(BF16)
        fpk = np.concatenate(
            [cos_l, sin_sw, np.zeros((128, KVER), np.float32)], axis=1
        ).astype(BF16)
        in_maps.append({
            "xT": xT.astype(BF16), "wpack": wpk, "fpack": fpk,
            "onesr": np.ones((1, 64), np.float32),
        })
    return in_maps


def _get_nc():
    if "nc" not in _CACHE:
        _CACHE["nc"] = _build_nc()
    return _CACHE["nc"]


def kernel(x, wq, wk, wv, wo, token_positions, _trace=False, _tmpdir=None):
    nc = _get_nc()
    in_maps = _host_prep(x, wq, wk, wv, wo, token_positions)
    res = run_bass_kernel_spmd(
        nc, in_maps, core_ids=list(range(8)), trace=_trace, tmpdir=_tmpdir
    )
    out = np.zeros((B, S, D), np.float32)
    for c in range(8):
        b = c // 4
        out[b] += res.results[c]["o"].astype(np.float32)
    if _trace:
        kernel._last_result = res
    return out
